# Initial kernel scaffold
#
"""GAT (graph attention) Bass kernel for Trainium2, data-parallel over batch.

Reference computation (per batch b):
    Wh   = hidden[b] @ W                            [S, F]
    e    = leaky_relu(Wh@a1 + (Wh@a2)^T, 0.2)       [S, S]   e[s,t] = Wh1[s]+Wh2[t]
    att  = softmax(where(adj>0.5, e, -9e15), axis over s)    (columns sum to 1)
    out  = elu(att^T-contracted: h[s,o] = sum_t att[s,t] Wh[t,o])

Sharding: batch b -> core b (8 cores). Host pre-marshals per-batch
transposed adjacency (adjT, f32), X^T (bf16), W (bf16), and the tiny
attention vectors Wh1 = X@(W@a1) (f32 row) / Wh2 = X@(W@a2) (f32,
column-chunked), which are O(S*F) host flops vs O(S^2*F) device flops.

Device pipeline per t-chunk c (layout [t=128 partitions, s=2048 free]):
    d  = (adjT >0.5) - 0.5          in {+.5,-.5}    (DVE, bf16 out)
    z  = (d*2e30) min Wh1bc         kept->Wh1[s], masked->-1e30 (DVE)
    lk = Prelu(z + Wh2[t], a=0.2)   (ACT, f32 out)
    p  = Exp(lk) -> bf16  (+free colsum accumulate) (ACT)
    rc = 1/colsum; Whp[c] = Wh[c]*rc[c]             (DVE)
    h[s-chunk] += p[c,s-chunk]^T @ Whp[c]           (PE, PSUM accumulate)
    elu: q=Exp(h) (ACT); out = max(h, (q-1) min 0)  (ACT+DVE)
"""
import numpy as np
import ml_dtypes
from contextlib import ExitStack

import concourse.tile as tile
from concourse import bacc, mybir
from concourse.bass_utils import run_bass_kernel_spmd

B, S, F = 8, 2048, 512
NCORES = 8
PC = 128                 # partition chunk
NC_T = S // PC           # 16 t-chunks
NC_S = S // PC           # 16 s-chunks
NK_I = F // PC           # 4 i-chunks (contraction for Wh)
ALPHA = 0.2
BIG = 2.0e30
WAVE_A = 8               # s-chunks accumulated during the t-chunk stream

bf16 = ml_dtypes.bfloat16

_cache = {}


def _build():
    nc = bacc.Bacc("TRN2", target_bir_lowering=False, debug=False,
                   num_devices=NCORES)
    adjT_d = nc.dram_tensor("adjT", [S, S], mybir.dt.float32,
                            kind="ExternalInput").ap()
    xT_d = nc.dram_tensor("xT", [F, S], mybir.dt.bfloat16,
                          kind="ExternalInput").ap()
    w_d = nc.dram_tensor("w", [F, F], mybir.dt.bfloat16,
                         kind="ExternalInput").ap()
    wh1_d = nc.dram_tensor("wh1", [1, S], mybir.dt.float32,
                           kind="ExternalInput").ap()
    wh2_d = nc.dram_tensor("wh2", [S, 1], mybir.dt.float32,
                           kind="ExternalInput").ap()
    out_d = nc.dram_tensor("h_out", [S, F], mybir.dt.float32,
                           kind="ExternalOutput").ap()

    with tile.TileContext(nc) as tc, ExitStack() as ctx:
        # ---- persistent SBUF tensors -------------------------------------
        const_pool = ctx.enter_context(tc.tile_pool(name="const", bufs=1))
        w_sb = const_pool.tile([PC, NK_I * F], mybir.dt.bfloat16)      # 4KB/p
        xT_sb = const_pool.tile([PC, NK_I * S], mybir.dt.bfloat16)     # 16KB/p
        wh1bc = const_pool.tile([PC, S], mybir.dt.bfloat16)            # 4KB/p
        wh2_sb = const_pool.tile([PC, NC_T], mybir.dt.float32)         # tiny
        wh_sb = const_pool.tile([PC, NC_T * F], mybir.dt.bfloat16)     # 16KB/p
        whp_sb = const_pool.tile([PC, NC_T * F], mybir.dt.bfloat16)    # 16KB/p
        p_sb = const_pool.tile([PC, NC_T * S], mybir.dt.bfloat16)      # 64KB/p
        cs_sb = const_pool.tile([PC, NC_T], mybir.dt.float32)
        rc_sb = const_pool.tile([PC, NC_T], mybir.dt.float32)

        nc.sync.dma_start(
            w_sb[:].rearrange("p (c o) -> p c o", o=F),
            w_d.rearrange("(c p) o -> p c o", p=PC))
        nc.sync.dma_start(
            xT_sb[:].rearrange("p (c s) -> p c s", s=S),
            xT_d.rearrange("(c p) s -> p c s", p=PC))
        nc.gpsimd.dma_start(wh1bc[:], wh1_d.partition_broadcast(PC))
        nc.sync.dma_start(
            wh2_sb[:].rearrange("p (c o) -> p c o", o=1),
            wh2_d.rearrange("(c p) o -> p c o", p=PC))

        # ---- Wh = X @ W  -> wh_sb (bf16) ---------------------------------
        with tc.tile_pool(name="whpsum", bufs=2, space="PSUM") as whps_pool:
            for m in range(NC_S):
                whps = whps_pool.tile([PC, F], mybir.dt.float32)
                for k in range(NK_I):
                    nc.tensor.matmul(
                        whps[:],
                        xT_sb[:, k * S + m * PC: k * S + (m + 1) * PC],
                        w_sb[:, k * F:(k + 1) * F],
                        start=(k == 0), stop=(k == NK_I - 1))
                nc.vector.tensor_copy(wh_sb[:, m * F:(m + 1) * F], whps[:])

        # ---- t-chunk stream: mask + leaky + exp + colsum + Wh scale ------
        adj_pool = ctx.enter_context(tc.tile_pool(name="adj", bufs=3))
        d_pool = ctx.enter_context(tc.tile_pool(name="d", bufs=2))
        z_pool = ctx.enter_context(tc.tile_pool(name="z", bufs=2))
        lk_pool = ctx.enter_context(tc.tile_pool(name="lk", bufs=2))

        wave_a_pool = ctx.enter_context(
            tc.tile_pool(name="wavea", bufs=WAVE_A, space="PSUM"))
        hps = [wave_a_pool.tile([PC, F], mybir.dt.float32)
               for _ in range(WAVE_A)]

        for c in range(NC_T):
            adj_t = adj_pool.tile([PC, S], mybir.dt.float32)
            nc.sync.dma_start(adj_t[:], adjT_d[c * PC:(c + 1) * PC, :])

            d_t = d_pool.tile([PC, S], mybir.dt.bfloat16)
            nc.vector.tensor_scalar(d_t[:], adj_t[:], 0.5, 0.5,
                                    mybir.AluOpType.is_gt,
                                    mybir.AluOpType.subtract)
            z_t = z_pool.tile([PC, S], mybir.dt.bfloat16)
            nc.vector.scalar_tensor_tensor(z_t[:], d_t[:], BIG, wh1bc[:],
                                           mybir.AluOpType.mult,
                                           mybir.AluOpType.min)
            lk_t = lk_pool.tile([PC, S], mybir.dt.float32)
            nc.scalar.activation(lk_t[:], z_t[:],
                                 mybir.ActivationFunctionType.Prelu,
                                 bias=wh2_sb[:, c:c + 1], scale=1.0,
                                 alpha=ALPHA)
            nc.scalar.activation(p_sb[:, c * S:(c + 1) * S], lk_t[:],
                                 mybir.ActivationFunctionType.Exp,
                                 accum_out=cs_sb[:, c:c + 1])
            nc.vector.reciprocal(rc_sb[:, c:c + 1], cs_sb[:, c:c + 1])
            nc.vector.tensor_scalar(whp_sb[:, c * F:(c + 1) * F],
                                    wh_sb[:, c * F:(c + 1) * F],
                                    rc_sb[:, c:c + 1], None,
                                    mybir.AluOpType.mult)
            # wave A: s-chunks 0..WAVE_A-1 accumulate as chunks arrive
            for m in range(WAVE_A):
                nc.tensor.matmul(
                    hps[m][:],
                    p_sb[:, c * S + m * PC: c * S + (m + 1) * PC],
                    whp_sb[:, c * F:(c + 1) * F],
                    start=(c == 0), stop=(c == NC_T - 1))

        # ---- ELU + store -------------------------------------------------
        q_pool = ctx.enter_context(tc.tile_pool(name="q", bufs=3))
        u_pool = ctx.enter_context(tc.tile_pool(name="u", bufs=3))
        o_pool = ctx.enter_context(tc.tile_pool(name="o", bufs=3))

        def elu_store(m, h_psum):
            q_t = q_pool.tile([PC, F], mybir.dt.float32)
            nc.scalar.activation(q_t[:], h_psum[:],
                                 mybir.ActivationFunctionType.Exp)
            u_t = u_pool.tile([PC, F], mybir.dt.float32)
            nc.vector.tensor_scalar(u_t[:], q_t[:], -1.0, 0.0,
                                    mybir.AluOpType.add,
                                    mybir.AluOpType.min)
            o_t = o_pool.tile([PC, F], mybir.dt.float32)
            nc.vector.tensor_tensor(o_t[:], h_psum[:], u_t[:],
                                    mybir.AluOpType.max)
            nc.sync.dma_start(out_d[m * PC:(m + 1) * PC, :], o_t[:])

        for m in range(WAVE_A):
            elu_store(m, hps[m])

        # ---- wave B: remaining s-chunks ---------------------------------
        with tc.tile_pool(name="waveb", bufs=4, space="PSUM") as wb_pool:
            for m in range(WAVE_A, NC_S):
                hb = wb_pool.tile([PC, F], mybir.dt.float32)
                for c in range(NC_T):
                    nc.tensor.matmul(
                        hb[:],
                        p_sb[:, c * S + m * PC: c * S + (m + 1) * PC],
                        whp_sb[:, c * F:(c + 1) * F],
                        start=(c == 0), stop=(c == NC_T - 1))
                elu_store(m, hb)

    nc.compile()
    return nc


def kernel(hidden_state, adjacent_matrix, W, a):
    hidden_state = np.asarray(hidden_state, dtype=np.float32)
    adjacent_matrix = np.asarray(adjacent_matrix, dtype=np.float32)
    W = np.asarray(W, dtype=np.float32)
    a = np.asarray(a, dtype=np.float32)

    if "nc" not in _cache:
        _cache["nc"] = _build()
    nc = _cache["nc"]

    # host marshaling (layout only + O(S*F) attention vectors)
    wa1 = W @ a[:F, :]                      # [F, 1]
    wa2 = W @ a[F:, :]                      # [F, 1]
    w_bf = W.astype(bf16)
    in_maps = []
    for b in range(NCORES):
        x = hidden_state[b]                                  # [S, F]
        in_maps.append({
            "adjT": np.ascontiguousarray(adjacent_matrix[b].T),
            "xT": np.ascontiguousarray(x.T).astype(bf16),
            "w": w_bf,
            "wh1": np.ascontiguousarray((x @ wa1).reshape(1, S)),
            "wh2": np.ascontiguousarray(x @ wa2).reshape(S, 1),
        })

    res = run_bass_kernel_spmd(nc, in_maps, core_ids=list(range(NCORES)))
    return np.stack([res.results[b]["h_out"] for b in range(NCORES)], axis=0)


# revision 8
# speedup vs baseline: 4.7843x; 4.7843x over previous
"""GAT (graph attention) Bass kernel for Trainium2, data-parallel over batch.

Reference computation (per batch b):
    Wh   = hidden[b] @ W                            [S, F]
    e    = leaky_relu(Wh@a1 + (Wh@a2)^T, 0.2)       [S, S]   e[s,t] = Wh1[s]+Wh2[t]
    att  = softmax(where(adj>0.5, e, -9e15), axis over s)    (columns sum to 1)
    out  = elu(att^T-contracted: h[s,o] = sum_t att[s,t] Wh[t,o])

Sharding: batch b -> core b (8 cores). Host pre-marshals per-batch
transposed adjacency (adjT, f32), X^T (bf16), W (bf16), and the tiny
attention vectors Wh1 = X@(W@a1) (f32 row) / Wh2 = X@(W@a2) (f32,
column-chunked), which are O(S*F) host flops vs O(S^2*F) device flops.

Device pipeline per t-chunk c (layout [t=128 partitions, s=2048 free]):
    d  = (adjT >0.5) - 0.5          in {+.5,-.5}    (DVE, bf16 out)
    z  = (d*2e30) min Wh1bc         kept->Wh1[s], masked->-1e30 (DVE)
    lk = Prelu(z + Wh2[t], a=0.2)   (ACT, f32 out)
    p  = Exp(lk) -> bf16  (+free colsum accumulate) (ACT)
    rc = 1/colsum; Whp[c] = Wh[c]*rc[c]             (DVE)
    h[s-chunk] += p[c,s-chunk]^T @ Whp[c]           (PE, PSUM accumulate)
    elu: q=Exp(h) (ACT); out = max(h, (q-1) min 0)  (ACT+DVE)
"""
import numpy as np
import ml_dtypes
from contextlib import ExitStack

import concourse.tile as tile
from concourse import bacc, mybir
from concourse.bass_utils import run_bass_kernel_spmd

B, S, F = 8, 2048, 512
NCORES = 8
PC = 128                 # partition chunk
NC_T = S // PC           # 16 t-chunks
NC_S = S // PC           # 16 s-chunks
NK_I = F // PC           # 4 i-chunks (contraction for Wh)
ALPHA = 0.2
BIG = 2.0e30
WAVE_A = 8               # s-chunks accumulated during the t-chunk stream

bf16 = ml_dtypes.bfloat16

_cache = {}


def _build(reps: int = 1):
    nc = bacc.Bacc("TRN2", target_bir_lowering=False, debug=False,
                   num_devices=NCORES)
    adjT_d = nc.dram_tensor("adjT", [S, S], mybir.dt.float32,
                            kind="ExternalInput").ap()
    xT_d = nc.dram_tensor("xT", [F, S], mybir.dt.bfloat16,
                          kind="ExternalInput").ap()
    w_d = nc.dram_tensor("w", [F, F], mybir.dt.bfloat16,
                         kind="ExternalInput").ap()
    wh1_d = nc.dram_tensor("wh1", [1, S], mybir.dt.float32,
                           kind="ExternalInput").ap()
    wh2_d = nc.dram_tensor("wh2", [S, 1], mybir.dt.float32,
                           kind="ExternalInput").ap()
    out_d = nc.dram_tensor("h_out", [S, F], mybir.dt.float32,
                           kind="ExternalOutput").ap()

    with tile.TileContext(nc) as tc, ExitStack() as outer_ctx:
        if reps > 1:
            loop = outer_ctx.enter_context(tc.For_i(0, reps, 1))
        ctx = outer_ctx
        # ---- persistent SBUF tensors -------------------------------------
        const_pool = ctx.enter_context(tc.tile_pool(name="const", bufs=1))
        w_sb = const_pool.tile([PC, NK_I * F], mybir.dt.bfloat16)      # 4KB/p
        xT_sb = const_pool.tile([PC, NK_I * S], mybir.dt.bfloat16)     # 16KB/p
        wh1bc = const_pool.tile([PC, S], mybir.dt.bfloat16)            # 4KB/p
        wh2_sb = const_pool.tile([PC, NC_T], mybir.dt.float32)         # tiny
        wh_sb = const_pool.tile([PC, NC_T * F], mybir.dt.bfloat16)     # 16KB/p
        whp_sb = const_pool.tile([PC, NC_T * F], mybir.dt.bfloat16)    # 16KB/p
        p_sb = const_pool.tile([PC, NC_T * S], mybir.dt.bfloat16)      # 64KB/p
        cs_sb = const_pool.tile([PC, NC_T], mybir.dt.float32)
        rc_sb = const_pool.tile([PC, NC_T], mybir.dt.float32)

        nc.sync.dma_start(
            w_sb[:].rearrange("p (c o) -> p c o", o=F),
            w_d.rearrange("(c p) o -> p c o", p=PC))
        nc.sync.dma_start(
            xT_sb[:].rearrange("p (c s) -> p c s", s=S),
            xT_d.rearrange("(c p) s -> p c s", p=PC))
        nc.gpsimd.dma_start(wh1bc[:], wh1_d.partition_broadcast(PC))
        nc.sync.dma_start(
            wh2_sb[:].rearrange("p (c o) -> p c o", o=1),
            wh2_d.rearrange("(c p) o -> p c o", p=PC))

        # ---- Wh = X @ W  -> wh_sb (bf16) ---------------------------------
        with tc.tile_pool(name="whpsum", bufs=2, space="PSUM") as whps_pool:
            for m in range(NC_S):
                whps = whps_pool.tile([PC, F], mybir.dt.float32)
                for k in range(NK_I):
                    nc.tensor.matmul(
                        whps[:],
                        xT_sb[:, k * S + m * PC: k * S + (m + 1) * PC],
                        w_sb[:, k * F:(k + 1) * F],
                        start=(k == 0), stop=(k == NK_I - 1))
                nc.vector.tensor_copy(wh_sb[:, m * F:(m + 1) * F], whps[:])

        # ---- t-chunk stream: mask + leaky + exp + colsum + Wh scale ------
        adj_pool = ctx.enter_context(tc.tile_pool(name="adj", bufs=3))
        d_pool = ctx.enter_context(tc.tile_pool(name="d", bufs=2))
        z_pool = ctx.enter_context(tc.tile_pool(name="z", bufs=2))
        lk_pool = ctx.enter_context(tc.tile_pool(name="lk", bufs=2))

        # ---- ELU + store pools (used by both waves) ----------------------
        q_pool = ctx.enter_context(tc.tile_pool(name="q", bufs=3))
        u_pool = ctx.enter_context(tc.tile_pool(name="u", bufs=3))
        o_pool = ctx.enter_context(tc.tile_pool(name="o", bufs=3))

        def elu_store(m, h_psum):
            q_t = q_pool.tile([PC, F], mybir.dt.float32, name=f"q{m}", tag="q")
            nc.scalar.activation(q_t[:], h_psum[:],
                                 mybir.ActivationFunctionType.Exp)
            u_t = u_pool.tile([PC, F], mybir.dt.float32, name=f"u{m}", tag="u")
            nc.vector.tensor_scalar(u_t[:], q_t[:], -1.0, 0.0,
                                    mybir.AluOpType.add,
                                    mybir.AluOpType.min)
            o_t = o_pool.tile([PC, F], mybir.dt.float32, name=f"o{m}", tag="o")
            nc.vector.tensor_tensor(o_t[:], h_psum[:], u_t[:],
                                    mybir.AluOpType.max)
            nc.sync.dma_start(out_d[m * PC:(m + 1) * PC, :], o_t[:])

        wave_a_ctx = tc.tile_pool(name="wavea", bufs=1, space="PSUM")
        wave_a_pool = wave_a_ctx.__enter__()
        hps = [wave_a_pool.tile([PC, F], mybir.dt.float32, tag=f"hps{m}",
                                name=f"hps{m}")
               for m in range(WAVE_A)]

        for c in range(NC_T):
            adj_t = adj_pool.tile([PC, S], mybir.dt.float32)
            nc.sync.dma_start(adj_t[:], adjT_d[c * PC:(c + 1) * PC, :])

            d_t = d_pool.tile([PC, S], mybir.dt.bfloat16)
            nc.vector.tensor_scalar(d_t[:], adj_t[:], 0.5, 0.5,
                                    mybir.AluOpType.is_gt,
                                    mybir.AluOpType.subtract)
            z_t = z_pool.tile([PC, S], mybir.dt.bfloat16)
            nc.vector.scalar_tensor_tensor(z_t[:], d_t[:], BIG, wh1bc[:],
                                           mybir.AluOpType.mult,
                                           mybir.AluOpType.min)
            lk_t = lk_pool.tile([PC, S], mybir.dt.float32)
            nc.scalar.activation(lk_t[:], z_t[:],
                                 mybir.ActivationFunctionType.Prelu,
                                 bias=wh2_sb[:, c:c + 1], scale=1.0,
                                 alpha=ALPHA)
            nc.scalar.activation(p_sb[:, c * S:(c + 1) * S], lk_t[:],
                                 mybir.ActivationFunctionType.Exp,
                                 accum_out=cs_sb[:, c:c + 1])
            nc.vector.reciprocal(rc_sb[:, c:c + 1], cs_sb[:, c:c + 1])
            nc.vector.tensor_scalar(whp_sb[:, c * F:(c + 1) * F],
                                    wh_sb[:, c * F:(c + 1) * F],
                                    rc_sb[:, c:c + 1], None,
                                    mybir.AluOpType.mult)
            # wave A: s-chunks 0..WAVE_A-1 accumulate as chunks arrive
            for m in range(WAVE_A):
                nc.tensor.matmul(
                    hps[m][:],
                    p_sb[:, c * S + m * PC: c * S + (m + 1) * PC],
                    whp_sb[:, c * F:(c + 1) * F],
                    start=(c == 0), stop=(c == NC_T - 1))

        # ---- ELU + store for wave A, then close its PSUM pool ------------
        for m in range(WAVE_A):
            elu_store(m, hps[m])
        wave_a_ctx.__exit__(None, None, None)

        # ---- wave B: remaining s-chunks ---------------------------------
        with tc.tile_pool(name="waveb", bufs=4, space="PSUM") as wb_pool:
            for m in range(WAVE_A, NC_S):
                hb = wb_pool.tile([PC, F], mybir.dt.float32)
                for c in range(NC_T):
                    nc.tensor.matmul(
                        hb[:],
                        p_sb[:, c * S + m * PC: c * S + (m + 1) * PC],
                        whp_sb[:, c * F:(c + 1) * F],
                        start=(c == 0), stop=(c == NC_T - 1))
                elu_store(m, hb)

    nc.compile()
    return nc


def kernel(hidden_state, adjacent_matrix, W, a):
    hidden_state = np.asarray(hidden_state, dtype=np.float32)
    adjacent_matrix = np.asarray(adjacent_matrix, dtype=np.float32)
    W = np.asarray(W, dtype=np.float32)
    a = np.asarray(a, dtype=np.float32)

    if "nc" not in _cache:
        _cache["nc"] = _build()
    nc = _cache["nc"]

    # host marshaling (layout only + O(S*F) attention vectors)
    wa1 = W @ a[:F, :]                      # [F, 1]
    wa2 = W @ a[F:, :]                      # [F, 1]
    w_bf = W.astype(bf16)
    in_maps = []
    for b in range(NCORES):
        x = hidden_state[b]                                  # [S, F]
        in_maps.append({
            "adjT": np.ascontiguousarray(adjacent_matrix[b].T),
            "xT": np.ascontiguousarray(x.T).astype(bf16),
            "w": w_bf,
            "wh1": np.ascontiguousarray((x @ wa1).reshape(1, S)),
            "wh2": np.ascontiguousarray(x @ wa2).reshape(S, 1),
        })

    res = run_bass_kernel_spmd(nc, in_maps, core_ids=list(range(NCORES)))
    return np.stack([res.results[b]["h_out"] for b in range(NCORES)], axis=0)
